# revision 5
# baseline (speedup 1.0000x reference)
"""DistMult edge-scoring kernel for Trainium2 (8 NeuronCores, SPMD).

score[j] = sum_d emb_A[a_idx[j], d] * k[d] * emb_B[b_idx[j], d]
for 9E pairs: E positive edges, 4E head-corrupted, 4E tail-corrupted.

Strategy (v6, transposed all-dense bf16 + PE reduce, single stream):
- HOST pre-gathers every pair's rows into one dense bf16 stream in a
  TRANSPOSED layout (d across the 128 partitions, pairs along the free
  dim), exploiting the repeat structure: per edge e only Ad=emb_A[ep0],
  Bd=emb_B[ep1]*k and the 8 corrupt rows stream in (10E rows total =
  256 MB vs 921 MB for the f32 gather baseline). k is folded host-side
  (appears exactly once per score: pos=<Ad,Bd>, head_i=<A[hb_i],Bd>,
  tail_i=<Bk[tb_i],Ad>).
- Per 128-edge group (1280 stream cols): DVE computes bf16 products
  with 2 tensor_tensor ops (broadcast APs share Ad/Bd across the 4
  corrupt slots; bf16 gets the DVE 2x mode). PE reduces over partitions
  with "flipped" matmuls: lhsT = one 128x128 product slot, rhs =
  ones[128,1], so the 128 scores of a slot land one-per-PSUM-partition.
  The otherwise idle Act engine evacuates psum[128,9] per group into
  the SBUF score tile.
- Engine budget per core: DMA ~93us (bound), DVE ~77us, PE ~55us,
  Act ~29us -> DMA-roofline bound (~98us modeled).
"""

import numpy as np

# problem constants
N_A = 100000
N_B = 100000
D = 128
E = 100000
NEG = 4
NCORES = 8

P = 128
EC = E // NCORES          # edges per core (12500)
G = -(-EC // P)           # groups of 128 edges per core (98)
PAD = G * P               # padded edges per core (12544)
BATCH = 2                 # groups per DMA batch
BUFS = 4
R = 9                     # scores per edge
W = 10 * P                # stream cols per group: [Ad|Bd|T0..T3|H0..H3]

_CACHED = {}


def _build_program():
    import concourse.tile as tile
    from concourse import bacc, mybir

    f32 = mybir.dt.float32
    bf16 = mybir.dt.bfloat16
    mult = mybir.AluOpType.mult

    nc = bacc.Bacc("TRN2", target_bir_lowering=False, debug=False,
                   num_devices=NCORES)
    x_d = nc.dram_tensor("x", [P, G * W], bf16, kind="ExternalInput").ap()
    # scores: [e-partition, g*9+j]; j 0-3: T_j, 4-7: H_{j-4}, 8: pos
    s_d = nc.dram_tensor("scores", [P, G * R], f32, kind="ExternalOutput").ap()

    with tile.TileContext(nc) as tc:
        with (
            tc.tile_pool(name="io", bufs=BUFS) as io_pool,
            tc.tile_pool(name="pr", bufs=3) as pr_pool,
            tc.tile_pool(name="ps", bufs=4, space="PSUM") as ps_pool,
            tc.tile_pool(name="on", bufs=1) as on_pool,
            tc.tile_pool(name="sc", bufs=1) as sc_pool,
        ):
            ones = on_pool.tile([P, 1], bf16)
            nc.vector.memset(ones[:], 1.0)
            sc = sc_pool.tile([P, G * R], f32)

            for b0 in range(0, G, BATCH):
                n = min(BATCH, G - b0)
                x = io_pool.tile([P, BATCH * W], bf16, tag="x")
                nc.sync.dma_start(x[:, :n * W], x_d[:, b0 * W:(b0 + n) * W])
                for j in range(n):
                    g = b0 + j
                    abg = x[:, j * W:j * W + 2 * P]
                    htg = x[:, j * W + 2 * P:(j + 1) * W]
                    prod = pr_pool.tile([P, R * P], bf16, tag="pr")

                    # corrupt products [d, c, i, e]: c=0 T_i*Ad, c=1 H_i*Bd
                    in0 = htg.rearrange("p (c i e) -> p c i e", c=2, i=4)
                    in1 = abg.rearrange("p (c one e) -> p c one e",
                                        c=2, one=1).broadcast_to([P, 2, 4, P])
                    out = prod[:, :8 * P].rearrange(
                        "p (c i e) -> p c i e", c=2, i=4)
                    nc.vector.tensor_tensor(out=out, in0=in0, in1=in1, op=mult)
                    # pos products -> slot 8
                    nc.vector.tensor_tensor(
                        out=prod[:, 8 * P:9 * P], in0=abg[:, :P],
                        in1=abg[:, P:2 * P], op=mult)

                    ps = ps_pool.tile([P, R], f32, tag="ps")
                    for r in range(R):
                        nc.tensor.matmul(ps[:, r:r + 1],
                                         prod[:, r * P:(r + 1) * P],
                                         ones[:], start=True, stop=True)
                    nc.scalar.copy(out=sc[:, g * R:(g + 1) * R], in_=ps[:])

            nc.sync.dma_start(s_d[:], sc[:])

    nc.compile()
    return nc


def _host_prep(emb_A, emb_B, rel_kernel, edge_pos, head_batch, tail_batch):
    """Pre-gather pair rows into per-core transposed dense bf16 streams."""
    import ml_dtypes
    bf16 = ml_dtypes.bfloat16

    kv = np.asarray(rel_kernel, dtype=np.float32)[0]
    A16 = np.asarray(emb_A, dtype=np.float32).astype(bf16)
    Bk16 = (np.asarray(emb_B, dtype=np.float32) * kv[None, :]).astype(bf16)
    ep = np.asarray(edge_pos, dtype=np.int64)
    hb = np.asarray(head_batch, dtype=np.int64)
    tb = np.asarray(tail_batch, dtype=np.int64)

    in_maps = []
    outpos_cores = []
    for c in range(NCORES):
        sl = slice(c * EC, (c + 1) * EC)
        e0 = np.zeros(PAD, np.int64)
        e1 = np.zeros(PAD, np.int64)
        hbp = np.zeros((PAD, NEG), np.int64)
        tbp = np.zeros((PAD, NEG), np.int64)
        e0[:EC], e1[:EC] = ep[0, sl], ep[1, sl]
        hbp[:EC], tbp[:EC] = hb[sl], tb[sl]

        # ab[d, g, c, e]: c=0 Ad, c=1 Bd
        abr = np.stack([A16[e0], Bk16[e1]], axis=1)      # [PAD, 2, D]
        ab3 = abr.reshape(G, P, 2, D).transpose(3, 0, 2, 1).reshape(P, G, 2 * P)
        # ht[d, g, c, i, e]: c=0 T_i (Bk16[tb]), c=1 H_i (A16[hb])
        tt4 = Bk16[tbp.reshape(-1)].reshape(G, P, NEG, D)
        hh4 = A16[hbp.reshape(-1)].reshape(G, P, NEG, D)
        htr = np.stack([tt4, hh4], axis=2)               # [G, e, c, i, d]
        ht3 = htr.transpose(4, 0, 2, 3, 1).reshape(P, G, 8 * P)
        x = np.ascontiguousarray(
            np.concatenate([ab3, ht3], axis=2).reshape(P, G * W))
        in_maps.append({"x": x})

        # flat scores idx = (g*R + r)*128 + p ; p = edge-in-group
        gg, rr, pp = np.meshgrid(np.arange(G), np.arange(R), np.arange(P),
                                 indexing="ij")
        el = gg * P + pp
        eg = c * EC + el
        valid = el < EC
        ov = np.where(
            rr == 8, eg,
            np.where(rr < 4, 5 * E + eg * NEG + rr,
                     E + eg * NEG + (rr - 4)))
        outpos_cores.append(np.where(valid, ov, -1).reshape(-1))
    return in_maps, outpos_cores


def kernel(emb_A, emb_B, rel_kernel, edge_pos, head_batch, tail_batch):
    from concourse.bass_utils import run_bass_kernel_spmd

    in_maps, outpos_cores = _host_prep(
        emb_A, emb_B, rel_kernel, edge_pos, head_batch, tail_batch)

    if "nc" not in _CACHED:
        _CACHED["nc"] = _build_program()
    nc = _CACHED["nc"]
    _CACHED["in_maps"] = in_maps
    _CACHED["plan"] = "v6"

    res = run_bass_kernel_spmd(nc, in_maps, core_ids=list(range(NCORES)))
    _CACHED["last_results"] = res

    out = np.empty(9 * E, dtype=np.float32)
    for c in range(NCORES):
        ov = outpos_cores[c]
        fv = res.results[c]["scores"].T.reshape(-1)
        m = ov >= 0
        out[ov[m]] = fv[m]
    return out


# revision 6
# speedup vs baseline: 2.8375x; 2.8375x over previous
"""DistMult edge-scoring kernel for Trainium2 (8 NeuronCores, SPMD).

score[j] = sum_d emb_A[a_idx[j], d] * k[d] * emb_B[b_idx[j], d]
for 9E pairs: E positive edges, 4E head-corrupted, 4E tail-corrupted.

Strategy (v6, transposed all-dense bf16 + PE reduce, single stream):
- HOST pre-gathers every pair's rows into one dense bf16 stream in a
  TRANSPOSED layout (d across the 128 partitions, pairs along the free
  dim), exploiting the repeat structure: per edge e only Ad=emb_A[ep0],
  Bd=emb_B[ep1]*k and the 8 corrupt rows stream in (10E rows total =
  256 MB vs 921 MB for the f32 gather baseline). k is folded host-side
  (appears exactly once per score: pos=<Ad,Bd>, head_i=<A[hb_i],Bd>,
  tail_i=<Bk[tb_i],Ad>).
- Per 128-edge group (1280 stream cols): DVE computes bf16 products
  with 2 tensor_tensor ops (broadcast APs share Ad/Bd across the 4
  corrupt slots; bf16 gets the DVE 2x mode). PE reduces over partitions
  with "flipped" matmuls: lhsT = one 128x128 product slot, rhs =
  ones[128,1], so the 128 scores of a slot land one-per-PSUM-partition.
  The otherwise idle Act engine evacuates psum[128,9] per group into
  the SBUF score tile.
- Engine budget per core: DMA ~93us (bound), DVE ~77us, PE ~55us,
  Act ~29us -> DMA-roofline bound (~98us modeled).
"""

import numpy as np

# problem constants
N_A = 100000
N_B = 100000
D = 128
E = 100000
NEG = 4
NCORES = 8

P = 128
EC = E // NCORES          # edges per core (12500)
G = -(-EC // P)           # groups of 128 edges per core (98)
PAD = G * P               # padded edges per core (12544)
BATCH = 1                 # groups per DMA batch
BUFS = 6
R = 9                     # scores per edge
W = 10 * P                # stream cols per group: [Ad|Bd|T0..T3|H0..H3]

_CACHED = {}


def _build_program():
    import concourse.tile as tile
    from concourse import bacc, mybir

    f32 = mybir.dt.float32
    bf16 = mybir.dt.bfloat16
    mult = mybir.AluOpType.mult

    nc = bacc.Bacc("TRN2", target_bir_lowering=False, debug=False,
                   num_devices=NCORES)
    x_d = nc.dram_tensor("x", [P, G * W], bf16, kind="ExternalInput").ap()
    # scores: [e-partition, g*9+j]; j 0-3: T_j, 4-7: H_{j-4}, 8: pos
    s_d = nc.dram_tensor("scores", [P, G * R], f32, kind="ExternalOutput").ap()

    with tile.TileContext(nc) as tc:
        with (
            tc.tile_pool(name="io", bufs=BUFS) as io_pool,
            tc.tile_pool(name="pr", bufs=3) as pr_pool,
            tc.tile_pool(name="ps", bufs=4, space="PSUM") as ps_pool,
            tc.tile_pool(name="on", bufs=1) as on_pool,
            tc.tile_pool(name="sc", bufs=1) as sc_pool,
        ):
            ones = on_pool.tile([P, 1], bf16)
            nc.vector.memset(ones[:], 1.0)
            sc = sc_pool.tile([P, G * R], f32)

            for b0 in range(0, G, BATCH):
                n = min(BATCH, G - b0)
                x = io_pool.tile([P, BATCH * W], bf16, tag="x")
                nc.sync.dma_start(x[:, :n * W], x_d[:, b0 * W:(b0 + n) * W])
                for j in range(n):
                    g = b0 + j
                    abg = x[:, j * W:j * W + 2 * P]
                    htg = x[:, j * W + 2 * P:(j + 1) * W]
                    prod = pr_pool.tile([P, R * P], bf16, tag="pr")

                    # corrupt products [d, c, i, e]: c=0 T_i*Ad, c=1 H_i*Bd
                    in0 = htg.rearrange("p (c i e) -> p c i e", c=2, i=4)
                    in1 = abg.rearrange("p (c one e) -> p c one e",
                                        c=2, one=1).broadcast_to([P, 2, 4, P])
                    out = prod[:, :8 * P].rearrange(
                        "p (c i e) -> p c i e", c=2, i=4)
                    nc.vector.tensor_tensor(out=out, in0=in0, in1=in1, op=mult)
                    # pos products -> slot 8
                    nc.vector.tensor_tensor(
                        out=prod[:, 8 * P:9 * P], in0=abg[:, :P],
                        in1=abg[:, P:2 * P], op=mult)

                    ps = ps_pool.tile([P, R], f32, tag="ps")
                    for r in range(R):
                        nc.tensor.matmul(ps[:, r:r + 1],
                                         prod[:, r * P:(r + 1) * P],
                                         ones[:], start=True, stop=True)
                    nc.scalar.copy(out=sc[:, g * R:(g + 1) * R], in_=ps[:])

            nc.sync.dma_start(s_d[:], sc[:])

    nc.compile()
    return nc


def _host_prep(emb_A, emb_B, rel_kernel, edge_pos, head_batch, tail_batch):
    """Pre-gather pair rows into per-core transposed dense bf16 streams."""
    import ml_dtypes
    bf16 = ml_dtypes.bfloat16

    kv = np.asarray(rel_kernel, dtype=np.float32)[0]
    A16 = np.asarray(emb_A, dtype=np.float32).astype(bf16)
    Bk16 = (np.asarray(emb_B, dtype=np.float32) * kv[None, :]).astype(bf16)
    ep = np.asarray(edge_pos, dtype=np.int64)
    hb = np.asarray(head_batch, dtype=np.int64)
    tb = np.asarray(tail_batch, dtype=np.int64)

    in_maps = []
    outpos_cores = []
    for c in range(NCORES):
        sl = slice(c * EC, (c + 1) * EC)
        e0 = np.zeros(PAD, np.int64)
        e1 = np.zeros(PAD, np.int64)
        hbp = np.zeros((PAD, NEG), np.int64)
        tbp = np.zeros((PAD, NEG), np.int64)
        e0[:EC], e1[:EC] = ep[0, sl], ep[1, sl]
        hbp[:EC], tbp[:EC] = hb[sl], tb[sl]

        # ab[d, g, c, e]: c=0 Ad, c=1 Bd
        abr = np.stack([A16[e0], Bk16[e1]], axis=1)      # [PAD, 2, D]
        ab3 = abr.reshape(G, P, 2, D).transpose(3, 0, 2, 1).reshape(P, G, 2 * P)
        # ht[d, g, c, i, e]: c=0 T_i (Bk16[tb]), c=1 H_i (A16[hb])
        tt4 = Bk16[tbp.reshape(-1)].reshape(G, P, NEG, D)
        hh4 = A16[hbp.reshape(-1)].reshape(G, P, NEG, D)
        htr = np.stack([tt4, hh4], axis=2)               # [G, e, c, i, d]
        ht3 = htr.transpose(4, 0, 2, 3, 1).reshape(P, G, 8 * P)
        x = np.ascontiguousarray(
            np.concatenate([ab3, ht3], axis=2).reshape(P, G * W))
        in_maps.append({"x": x})

        # flat scores idx = (g*R + r)*128 + p ; p = edge-in-group
        gg, rr, pp = np.meshgrid(np.arange(G), np.arange(R), np.arange(P),
                                 indexing="ij")
        el = gg * P + pp
        eg = c * EC + el
        valid = el < EC
        ov = np.where(
            rr == 8, eg,
            np.where(rr < 4, 5 * E + eg * NEG + rr,
                     E + eg * NEG + (rr - 4)))
        outpos_cores.append(np.where(valid, ov, -1).reshape(-1))
    return in_maps, outpos_cores


def kernel(emb_A, emb_B, rel_kernel, edge_pos, head_batch, tail_batch):
    from concourse.bass_utils import run_bass_kernel_spmd

    in_maps, outpos_cores = _host_prep(
        emb_A, emb_B, rel_kernel, edge_pos, head_batch, tail_batch)

    if "nc" not in _CACHED:
        _CACHED["nc"] = _build_program()
    nc = _CACHED["nc"]
    _CACHED["in_maps"] = in_maps
    _CACHED["plan"] = "v6"

    res = run_bass_kernel_spmd(nc, in_maps, core_ids=list(range(NCORES)))
    _CACHED["last_results"] = res

    out = np.empty(9 * E, dtype=np.float32)
    for c in range(NCORES):
        ov = outpos_cores[c]
        fv = res.results[c]["scores"].T.reshape(-1)
        m = ov >= 0
        out[ov[m]] = fv[m]
    return out
